# revision 7
# baseline (speedup 1.0000x reference)
"""AdaptiveAttention Trainium2 kernel (8 NeuronCores, SPMD, no collectives).

Sharding: core c -> batch b=c//4, head group g=c%4 (4 of 16 heads).
Each core computes its 4 heads' Q/K/V projections (256 of 1024 channels)
for its batch, full attention for those heads in transposed layout
(scoresT[k,q] = K Q^T so softmax sums come free from a ones-row fused
into the attn@V matmul), writes attn_weights^T in bf16 plus a partial
output projection. Host sums the 4 partials per batch, adds bo, and
transposes attn weights back.
"""

import os
import sys

for _p in ("/opt/trn_rl_repo", "/root/.axon_site/_ro/trn_rl_repo"):
    if os.path.isdir(_p) and _p not in sys.path:
        sys.path.insert(0, _p)

import ml_dtypes
import numpy as np

import concourse.bass as bass
import concourse.tile as tile
from concourse import mybir
from concourse.bass_utils import run_bass_kernel_spmd

# ---------------------------------------------------------------------------
# Workaround: walrus rejects the TileContext tail drain when it carries >2
# sem waits ("Too many sync wait commands"). Emit standalone 1-wait SP
# instructions instead.
from bass_rust import ScopedClock

_WAIT_MODE = {"sem-ge-imm": "sem-ge", "sem-eq-imm": "sem-eq"}


def _patched_drain_and_barrier(self, tick_clock, wait_clock):
    nc = self.nc
    drain_inst = nc.sync.drain()
    wait_clock.add_sem_waits(
        drain_inst.ins, ScopedClock({None: tick_clock.global_clock})
    )
    si = drain_inst.ins.sync_info
    if si is not None and len(si.on_wait) > 1:
        waits = list(si.on_wait)
        si.on_wait = []
        by_num = {h.num: h for h in self.sems.allocated().values()}
        for w in waits:
            h = by_num.get(w.id)
            assert h is not None, f"no handle for sem {w.id} ({w.ant_name})"
            nc.sync.wait_op(h, w.wait_value, _WAIT_MODE[w.wait_mode])
    nc.all_engine_barrier()
    popped = nc._tile_sem_poison_stack.pop()
    assert popped is self._sem_poison
    nc.clear_and_free_semaphores(list(self.sems.allocated().values()))
    nc.all_engine_barrier()


tile.TileContext._drain_and_barrier = _patched_drain_and_barrier

_MAX_WAITS = {}
_DEFAULT_MAX_WAITS = 1
_split_ctr = [0]


def _split_excess_waits(nc):
    """Hoist sync waits beyond the per-instruction-type hardware limit onto
    standalone same-engine wait instructions inserted just before."""
    for fn in nc.m.functions:
        for bb in fn.blocks:
            il = bb.instructions
            new_list = []
            changed = False
            for inst in il:
                si = inst.sync_info
                maxw = _MAX_WAITS.get(type(inst).__name__, _DEFAULT_MAX_WAITS)
                if si is not None and len(si.on_wait) > maxw:
                    waits = list(si.on_wait)
                    extra, keep = waits[:-maxw], waits[-maxw:]
                    for j in range(0, len(extra), 2):
                        _split_ctr[0] += 1
                        w = mybir.InstEventSemaphore(
                            name=f"xw-{_split_ctr[0]}", ins=[], outs=[]
                        )
                        w.engine = inst.engine
                        w.sync_info = mybir.SyncInfo(
                            on_wait=extra[j : j + 2], on_update=[]
                        )
                        new_list.append(w)
                    si.on_wait = keep
                    changed = True
                new_list.append(inst)
            if changed:
                bb.instructions = new_list
# ---------------------------------------------------------------------------

B, S, HS, NH, NP = 2, 2048, 1024, 16, 8
HD = HS // NH  # 64
SCALE = 1.0 / np.sqrt(HD)
N_CORES = 8
GH = 4  # heads per core
DL = GH * HD  # 256 local channels
CT = HS // 128  # 8 contraction tiles
QT = 4  # q tiles of 512
KT = S // 128  # 16 k partition tiles
BF16 = mybir.dt.bfloat16
F32 = mybir.dt.float32

_cache = {}


def _build(apply_mask: bool) -> bass.Bass:
    nc = bass.Bass()

    xq = nc.declare_dram_parameter("xq", [128, CT, S], BF16, isOutput=False)
    xk = nc.declare_dram_parameter("xk", [128, CT, S], BF16, isOutput=False)
    xv = nc.declare_dram_parameter("xv", [128, CT, S], BF16, isOutput=False)
    wq = nc.declare_dram_parameter("wq", [128, CT, DL], BF16, isOutput=False)
    wk = nc.declare_dram_parameter("wk", [128, CT, DL], BF16, isOutput=False)
    wv = nc.declare_dram_parameter("wv", [128, CT, DL], BF16, isOutput=False)
    wo = nc.declare_dram_parameter("wo", [128, 2, HS], BF16, isOutput=False)
    bq_c = nc.declare_dram_parameter("bq_c", [128, 2], F32, isOutput=False)
    bk_c = nc.declare_dram_parameter("bk_c", [128, 2], F32, isOutput=False)
    bv_b = nc.declare_dram_parameter("bv_b", [128, 4, 64], F32, isOutput=False)
    wp1 = nc.declare_dram_parameter("wp1", [128, CT, 512], BF16, isOutput=False)
    bp1_c = nc.declare_dram_parameter("bp1_c", [128, 4], F32, isOutput=False)
    wp2 = nc.declare_dram_parameter("wp2", [128, 4, NP], BF16, isOutput=False)
    bp2_c = nc.declare_dram_parameter("bp2_c", [NP, 1], F32, isOutput=False)
    patm = nc.declare_dram_parameter("patm", [NP, GH], BF16, isOutput=False)
    if apply_mask:
        mk = nc.declare_dram_parameter("mk", [128, KT, S], BF16, isOutput=False)

    attn_ext = nc.declare_dram_parameter("attn_t", [GH, S, S], BF16, isOutput=True)
    out_ext = nc.declare_dram_parameter("out_part", [S, HS], F32, isOutput=True)

    with tile.TileContext(nc) as tc:
        from contextlib import ExitStack

        with ExitStack() as outer:
            const = outer.enter_context(tc.tile_pool(name="const", bufs=1))
            psA = outer.enter_context(tc.tile_pool(name="psA", bufs=2, space="PSUM"))
            psB = outer.enter_context(tc.tile_pool(name="psB", bufs=2, space="PSUM"))
            psCT = outer.enter_context(tc.tile_pool(name="psCT", bufs=2, space="PSUM"))

            # ---- persistent SBUF tensors ----
            wq_sb = const.tile([128, CT, DL], BF16, tag="wq")
            wk_sb = const.tile([128, CT, DL], BF16, tag="wk")
            wv_sb = const.tile([128, CT, DL], BF16, tag="wv")
            wo_sb = const.tile([128, 2, HS], BF16, tag="wo")
            wp1_sb = const.tile([128, CT, 512], BF16, tag="wp1")
            wp2_sb = const.tile([128, 4, NP], BF16, tag="wp2")
            patm_sb = const.tile([NP, GH], BF16, tag="patm")
            bq_sb = const.tile([128, 2], F32, tag="bq")
            bk_sb = const.tile([128, 2], F32, tag="bk")
            bv_sb = const.tile([128, 4, 64], F32, tag="bv")
            bp1_sb = const.tile([128, 4], F32, tag="bp1")
            bp2_sb = const.tile([NP, 1], F32, tag="bp2")
            qt_sb = const.tile([128, 2, S], BF16, tag="qt")
            kt_sb = const.tile([128, 2, S], BF16, tag="kt")
            vaug_sb = const.tile([128, KT, GH, HD + 1], BF16, tag="vaug")
            ctn_sb = const.tile([128, 2, S], BF16, tag="ctn")
            ones128 = const.tile([1, 128], BF16, tag="ones128")
            pat_bc = const.tile([128, GH], F32, tag="patbc")

            nc.sync.dma_start(out=wq_sb[:], in_=wq[:])
            nc.sync.dma_start(out=wk_sb[:], in_=wk[:])
            nc.sync.dma_start(out=wv_sb[:], in_=wv[:])
            nc.sync.dma_start(out=wo_sb[:], in_=wo[:])
            nc.sync.dma_start(out=wp1_sb[:], in_=wp1[:])
            nc.sync.dma_start(out=wp2_sb[:], in_=wp2[:])
            nc.sync.dma_start(out=patm_sb[:], in_=patm[:])
            nc.sync.dma_start(out=bq_sb[:], in_=bq_c[:])
            nc.sync.dma_start(out=bk_sb[:], in_=bk_c[:])
            nc.sync.dma_start(out=bv_sb[:], in_=bv_b[:])
            nc.sync.dma_start(out=bp1_sb[:], in_=bp1_c[:])
            nc.sync.dma_start(out=bp2_sb[:], in_=bp2_c[:])
            nc.vector.memset(ones128[:], 1.0)
            nc.vector.memset(vaug_sb[:, :, :, HD : HD + 1], 1.0)

            with ExitStack() as xph:
                xpool = xph.enter_context(tc.tile_pool(name="xpool", bufs=1))
                xq_sb = xpool.tile([128, CT, S], BF16, tag="xq")
                xk_sb = xpool.tile([128, CT, S], BF16, tag="xk")
                xv_sb = xpool.tile([128, CT, S], BF16, tag="xv")
                nc.sync.dma_start(out=xq_sb[:], in_=xq[:])
                nc.sync.dma_start(out=xk_sb[:], in_=xk[:])
                nc.sync.dma_start(out=xv_sb[:], in_=xv[:])

                sm = xph.enter_context(tc.tile_pool(name="sel", bufs=1))

                # ---- pattern selector (tiny) ----
                mq_f = sm.tile([128, CT], F32, tag="mqf")
                for ct in range(CT):
                    nc.vector.tensor_reduce(
                        out=mq_f[:, ct : ct + 1],
                        in_=xq_sb[:, ct, :],
                        axis=mybir.AxisListType.X,
                        op=mybir.AluOpType.add,
                    )
                mq_bf = sm.tile([128, CT], BF16, tag="mqb")
                nc.vector.tensor_copy(out=mq_bf[:], in_=mq_f[:])

                h_sb = sm.tile([128, 4], BF16, tag="hsb")
                for jt in range(4):
                    h_ps = psB.tile([128, 1], F32, tag="b")
                    for ct in range(CT):
                        nc.tensor.matmul(
                            out=h_ps[:],
                            lhsT=wp1_sb[:, ct, jt * 128 : (jt + 1) * 128],
                            rhs=mq_bf[:, ct : ct + 1],
                            start=(ct == 0),
                            stop=(ct == CT - 1),
                        )
                    nc.scalar.activation(
                        out=h_sb[:, jt : jt + 1],
                        in_=h_ps[:],
                        func=mybir.ActivationFunctionType.Relu,
                        bias=bp1_sb[:, jt : jt + 1],
                    )

                lg_ps = psB.tile([NP, 1], F32, tag="b")
                for jt in range(4):
                    nc.tensor.matmul(
                        out=lg_ps[:],
                        lhsT=wp2_sb[:, jt, :],
                        rhs=h_sb[:, jt : jt + 1],
                        start=(jt == 0),
                        stop=(jt == 3),
                    )
                explg = sm.tile([NP, 1], BF16, tag="explg")
                nc.scalar.activation(
                    out=explg[:],
                    in_=lg_ps[:],
                    func=mybir.ActivationFunctionType.Exp,
                    bias=bp2_sb[:],
                )
                ones_np = sm.tile([NP, 1], BF16, tag="onesnp")
                nc.vector.memset(ones_np[:], 1.0)
                den_ps = psB.tile([1, 1], F32, tag="b")
                nc.tensor.matmul(
                    out=den_ps[:], lhsT=ones_np[:],
                    rhs=explg[:], start=True, stop=True,
                )
                rec_f = sm.tile([1, 1], F32, tag="recf")
                nc.vector.reciprocal(out=rec_f[:], in_=den_ps[:])
                rec_b = sm.tile([1, 1], BF16, tag="recb")
                nc.vector.tensor_copy(out=rec_b[:], in_=rec_f[:])
                bc8_ps = psB.tile([NP, 1], F32, tag="b")
                nc.tensor.matmul(
                    out=bc8_ps[:], lhsT=ones128[:, 0:NP], rhs=rec_b[:],
                    start=True, stop=True,
                )
                pw = sm.tile([NP, 1], BF16, tag="pw")
                nc.vector.tensor_tensor(
                    out=pw[:], in0=explg[:], in1=bc8_ps[:],
                    op=mybir.AluOpType.mult,
                )
                pat4_ps = psB.tile([1, GH], F32, tag="b")
                nc.tensor.matmul(
                    out=pat4_ps[:], lhsT=pw[:], rhs=patm_sb[:],
                    start=True, stop=True,
                )
                pat4_b = sm.tile([1, GH], BF16, tag="pat4")
                nc.vector.tensor_copy(out=pat4_b[:], in_=pat4_ps[:])
                for h in range(GH):
                    pbc_ps = psB.tile([128, 1], F32, tag="b")
                    nc.tensor.matmul(
                        out=pbc_ps[:], lhsT=ones128[:], rhs=pat4_b[:, h : h + 1],
                        start=True, stop=True,
                    )
                    nc.vector.tensor_copy(out=pat_bc[:, h : h + 1], in_=pbc_ps[:])

                # ---- projections ----
                # V first (feeds attn@V); out[tok,128..] = x^T tile as lhsT
                for tk in range(KT):
                    v_ps = psB.tile([128, DL], F32, tag="b")
                    for ct in range(CT):
                        nc.tensor.matmul(
                            out=v_ps[:],
                            lhsT=xv_sb[:, ct, tk * 128 : (tk + 1) * 128],
                            rhs=wv_sb[:, ct, :],
                            start=(ct == 0),
                            stop=(ct == CT - 1),
                        )
                    nc.vector.tensor_tensor(
                        out=vaug_sb[:, tk, :, 0:HD],
                        in0=v_ps[:].rearrange("p (h d) -> p h d", h=GH),
                        in1=bv_sb[:],
                        op=mybir.AluOpType.add,
                    )

                # Q^T, K^T: [dlocal, tokens]
                for pt in range(2):
                    for tt in range(QT):
                        q_ps = psA.tile([128, 512], F32, tag="a")
                        for ct in range(CT):
                            nc.tensor.matmul(
                                out=q_ps[:],
                                lhsT=wq_sb[:, ct, pt * 128 : (pt + 1) * 128],
                                rhs=xq_sb[:, ct, tt * 512 : (tt + 1) * 512],
                                start=(ct == 0),
                                stop=(ct == CT - 1),
                            )
                        nc.vector.tensor_scalar(
                            out=qt_sb[:, pt, tt * 512 : (tt + 1) * 512],
                            in0=q_ps[:],
                            scalar1=bq_sb[:, pt : pt + 1],
                            scalar2=None,
                            op0=mybir.AluOpType.add,
                        )
                        k_ps = psA.tile([128, 512], F32, tag="a")
                        for ct in range(CT):
                            nc.tensor.matmul(
                                out=k_ps[:],
                                lhsT=wk_sb[:, ct, pt * 128 : (pt + 1) * 128],
                                rhs=xk_sb[:, ct, tt * 512 : (tt + 1) * 512],
                                start=(ct == 0),
                                stop=(ct == CT - 1),
                            )
                        nc.vector.tensor_scalar(
                            out=kt_sb[:, pt, tt * 512 : (tt + 1) * 512],
                            in0=k_ps[:],
                            scalar1=bk_sb[:, pt : pt + 1],
                            scalar2=None,
                            op0=mybir.AluOpType.add,
                        )

            # ---- attention ----
            with ExitStack() as aph:
                if apply_mask:
                    mpool = aph.enter_context(tc.tile_pool(name="mpool", bufs=1))
                    mk_sb = mpool.tile([128, KT, S], BF16, tag="mk")
                    nc.sync.dma_start(out=mk_sb[:], in_=mk[:])
                epool = aph.enter_context(tc.tile_pool(name="epool", bufs=2))
                stpool = aph.enter_context(tc.tile_pool(name="stpool", bufs=2))
                bcpool = aph.enter_context(tc.tile_pool(name="bcpool", bufs=2))
                opool = aph.enter_context(tc.tile_pool(name="opool", bufs=2))

                for hp in range(2):  # head pairs: rows 0-63 / 64-127 of QT/KT
                    for qt in range(QT):
                        exps = []
                        cts = []
                        for hh in range(2):
                            h = 2 * hp + hh
                            r0 = hh * HD
                            exp_sb = epool.tile([128, KT, 512], BF16, tag="exp")
                            ct_ps = psCT.tile([HD + 1, 512], F32, tag="ct")
                            exps.append(exp_sb)
                            cts.append(ct_ps)
                        for kj in range(KT // 2):
                            for hh in range(2):
                                h = 2 * hp + hh
                                r0 = hh * HD
                                sc_ps = psA.tile([128, 1024], F32, tag="a")
                                for half in range(2):
                                    kt = 2 * kj + half
                                    nc.tensor.matmul(
                                        out=sc_ps[:, half * 512 : (half + 1) * 512],
                                        lhsT=kt_sb[
                                            r0 : r0 + HD, hp, kt * 128 : (kt + 1) * 128
                                        ],
                                        rhs=qt_sb[
                                            r0 : r0 + HD, hp,
                                            qt * 512 : (qt + 1) * 512,
                                        ],
                                        start=True,
                                        stop=True,
                                    )
                                nc.scalar.activation(
                                    out=exps[hh][:, 2 * kj : 2 * kj + 2, :],
                                    in_=sc_ps[:].rearrange(
                                        "p (a b) -> p a b", a=2
                                    ),
                                    func=mybir.ActivationFunctionType.Exp,
                                    scale=pat_bc[:, h : h + 1],
                                )
                                if apply_mask:
                                    nc.vector.tensor_tensor(
                                        out=exps[hh][:, 2 * kj : 2 * kj + 2, :],
                                        in0=exps[hh][:, 2 * kj : 2 * kj + 2, :],
                                        in1=mk_sb[
                                            :, 2 * kj : 2 * kj + 2,
                                            qt * 512 : (qt + 1) * 512,
                                        ],
                                        op=mybir.AluOpType.mult,
                                    )
                                for half in range(2):
                                    kt = 2 * kj + half
                                    nc.tensor.matmul(
                                        out=cts[hh][:],
                                        lhsT=vaug_sb[:, kt, 2 * hp + hh, :],
                                        rhs=exps[hh][:, kt, :],
                                        start=(kt == 0),
                                        stop=(kt == KT - 1),
                                    )
                        for hh in range(2):
                            h = 2 * hp + hh
                            ct_ps = cts[hh]
                            exp_sb = exps[hh]
                            rec_f = bcpool.tile([1, 512], F32, tag="recf")
                            nc.vector.reciprocal(
                                out=rec_f[:], in_=ct_ps[HD : HD + 1, :]
                            )
                            rec_b = bcpool.tile([1, 512], BF16, tag="recb")
                            nc.vector.tensor_copy(out=rec_b[:], in_=rec_f[:])
                            bc_ps = psB.tile([128, 512], F32, tag="b")
                            nc.tensor.matmul(
                                out=bc_ps[:], lhsT=ones128[:], rhs=rec_b[:],
                                start=True, stop=True,
                            )
                            bc_bf = bcpool.tile([128, 512], BF16, tag="bcbf")
                            nc.scalar.copy(out=bc_bf[:], in_=bc_ps[:])
                            # context normalize -> ctn[dlocal, tokens]
                            nc.vector.tensor_tensor(
                                out=ctn_sb[
                                    hh * HD : (hh + 1) * HD, hp,
                                    qt * 512 : (qt + 1) * 512,
                                ],
                                in0=ct_ps[0:HD, :],
                                in1=bc_bf[0:HD, :],
                                op=mybir.AluOpType.mult,
                            )
                            # attn normalize + stage + DMA out
                            attn_st = stpool.tile([128, KT, 512], BF16, tag="st")
                            for kt in range(KT):
                                nc.vector.tensor_tensor(
                                    out=attn_st[:, kt, :],
                                    in0=exp_sb[:, kt, :],
                                    in1=bc_bf[:],
                                    op=mybir.AluOpType.mult,
                                )
                            nc.sync.dma_start(
                                out=attn_ext[
                                    h, :, qt * 512 : (qt + 1) * 512
                                ].rearrange("(kt p) q -> p kt q", p=128),
                                in_=attn_st[:],
                            )

                # ---- output projection (partial) ----
                for tt in range(KT):
                    ostage = opool.tile([128, HS], F32, tag="ost")
                    for oh in range(2):
                        o_ps = psB.tile([128, 512], F32, tag="b")
                        for pt in range(2):
                            nc.tensor.matmul(
                                out=o_ps[:],
                                lhsT=ctn_sb[:, pt, tt * 128 : (tt + 1) * 128],
                                rhs=wo_sb[:, pt, oh * 512 : (oh + 1) * 512],
                                start=(pt == 0),
                                stop=(pt == 1),
                            )
                        nc.scalar.copy(
                            out=ostage[:, oh * 512 : (oh + 1) * 512], in_=o_ps[:]
                        )
                    nc.sync.dma_start(
                        out=out_ext[tt * 128 : (tt + 1) * 128, :], in_=ostage[:]
                    )

    _split_excess_waits(nc)
    return nc


def _prep_inputs(query, key, value, attention_mask, Wq, bq, Wk, bk, Wv, bv,
                 Wo, bo, Wp1, bp1, Wp2, bp2, patterns, apply_mask):
    bf = ml_dtypes.bfloat16
    f32 = np.float32

    def arr_x(x_b):  # [S, HS] -> [128, CT, S] bf16 of x_b.T
        t = np.ascontiguousarray(x_b.T.reshape(CT, 128, S).transpose(1, 0, 2))
        return t.astype(bf)

    def arr_w(W, g):  # [HS, HS] -> [128, CT, DL] col slice
        Wg = W[:, g * DL : (g + 1) * DL]
        return np.ascontiguousarray(
            Wg.reshape(CT, 128, DL).transpose(1, 0, 2)
        ).astype(bf)

    xqs = [arr_x(np.asarray(query[b], f32)) for b in range(B)]
    xks = [arr_x(np.asarray(key[b], f32)) for b in range(B)]
    xvs = [arr_x(np.asarray(value[b], f32)) for b in range(B)]
    wp1_a = np.ascontiguousarray(
        (np.asarray(Wp1, f32) / S).reshape(CT, 128, 512).transpose(1, 0, 2)
    ).astype(bf)
    bp1_a = np.ascontiguousarray(np.asarray(bp1, f32).reshape(4, 128).T)
    wp2_a = np.ascontiguousarray(
        np.asarray(Wp2, f32).reshape(4, 128, NP).transpose(1, 0, 2)
    ).astype(bf)
    bp2_a = np.asarray(bp2, f32).reshape(NP, 1).copy()
    patterns_r = np.asarray(patterns, f32).reshape(NP, NH)

    if apply_mask:
        mks = []
        for b in range(B):
            mt = np.asarray(attention_mask[b], f32).T  # [k, q]
            mks.append(
                np.ascontiguousarray(
                    mt.reshape(KT, 128, S).transpose(1, 0, 2)
                ).astype(bf)
            )

    in_maps = []
    for c in range(N_CORES):
        b, g = c // 4, c % 4
        im = {
            "xq": xqs[b], "xk": xks[b], "xv": xvs[b],
            "wq": arr_w(np.asarray(Wq, f32), g),
            "wk": arr_w(np.asarray(Wk, f32), g),
            "wv": arr_w(np.asarray(Wv, f32), g),
            "wo": np.ascontiguousarray(
                np.asarray(Wo, f32)[g * DL : (g + 1) * DL, :]
                .reshape(2, 128, HS).transpose(1, 0, 2)
            ).astype(bf),
            "bq_c": np.ascontiguousarray(
                np.asarray(bq, f32)[g * DL : (g + 1) * DL].reshape(2, 128).T
            ),
            "bk_c": np.ascontiguousarray(
                np.asarray(bk, f32)[g * DL : (g + 1) * DL].reshape(2, 128).T
            ),
            "bv_b": np.ascontiguousarray(
                np.broadcast_to(
                    np.asarray(bv, f32)[g * DL : (g + 1) * DL].reshape(1, 4, 64),
                    (128, 4, 64),
                )
            ),
            "wp1": wp1_a, "bp1_c": bp1_a, "wp2": wp2_a, "bp2_c": bp2_a,
            "patm": np.ascontiguousarray(
                patterns_r[:, g * GH : (g + 1) * GH] * SCALE
            ).astype(bf),
        }
        if apply_mask:
            im["mk"] = mks[b]
        in_maps.append(im)
    return in_maps


def kernel(**inputs):
    attention_mask = np.asarray(inputs["attention_mask"])
    apply_mask = not bool(attention_mask.all())

    if apply_mask not in _cache:
        _cache[apply_mask] = _build(apply_mask)
    nc = _cache[apply_mask]

    in_maps = _prep_inputs(
        inputs["query"], inputs["key"], inputs["value"], attention_mask,
        inputs["Wq"], inputs["bq"], inputs["Wk"], inputs["bk"],
        inputs["Wv"], inputs["bv"], inputs["Wo"], inputs["bo"],
        inputs["Wp1"], inputs["bp1"], inputs["Wp2"], inputs["bp2"],
        inputs["patterns"], apply_mask,
    )

    res = run_bass_kernel_spmd(nc, in_maps, core_ids=list(range(N_CORES)))

    f32 = np.float32
    output = np.zeros((B, S, HS), f32)
    attn = np.empty((B, NH, S, S), f32)
    for c in range(N_CORES):
        b, g = c // 4, c % 4
        r = res.results[c]
        output[b] += np.asarray(r["out_part"], f32)
        at = np.asarray(r["attn_t"]).astype(f32)  # [GH, k, q]
        for h in range(GH):
            attn[b, g * GH + h] = at[h].T
    output += np.asarray(inputs["bo"], f32)[None, None, :]
    return output, attn


# revision 17
# speedup vs baseline: 27.3211x; 27.3211x over previous
"""AdaptiveAttention Trainium2 kernel (8 NeuronCores, SPMD, no collectives).

Sharding: core c -> batch b=c//4, head group g=c%4 (4 of 16 heads).
Each core computes its 4 heads' Q/K/V projections (256 of 1024 channels)
for its batch, full attention for those heads in transposed layout
(scoresT[k,q] = K Q^T so softmax sums come free from a ones-row fused
into the attn@V matmul), writes attn_weights^T in bf16 plus a partial
output projection. Host sums the 4 partials per batch, adds bo, and
transposes attn weights back.
"""

import os
import sys

for _p in ("/opt/trn_rl_repo", "/root/.axon_site/_ro/trn_rl_repo"):
    if os.path.isdir(_p) and _p not in sys.path:
        sys.path.insert(0, _p)

import ml_dtypes
import numpy as np

import concourse.bass as bass
import concourse.tile as tile
from concourse import mybir
from concourse.bass_utils import run_bass_kernel_spmd

# ---------------------------------------------------------------------------
# Workaround: walrus rejects the TileContext tail drain when it carries >2
# sem waits ("Too many sync wait commands"). Emit standalone 1-wait SP
# instructions instead.
from bass_rust import ScopedClock

_WAIT_MODE = {"sem-ge-imm": "sem-ge", "sem-eq-imm": "sem-eq"}


def _patched_drain_and_barrier(self, tick_clock, wait_clock):
    nc = self.nc
    drain_inst = nc.sync.drain()
    wait_clock.add_sem_waits(
        drain_inst.ins, ScopedClock({None: tick_clock.global_clock})
    )
    si = drain_inst.ins.sync_info
    if si is not None and len(si.on_wait) > 1:
        waits = list(si.on_wait)
        si.on_wait = []
        by_num = {h.num: h for h in self.sems.allocated().values()}
        for w in waits:
            h = by_num.get(w.id)
            assert h is not None, f"no handle for sem {w.id} ({w.ant_name})"
            nc.sync.wait_op(h, w.wait_value, _WAIT_MODE[w.wait_mode])
    nc.all_engine_barrier()
    popped = nc._tile_sem_poison_stack.pop()
    assert popped is self._sem_poison
    nc.clear_and_free_semaphores(list(self.sems.allocated().values()))
    nc.all_engine_barrier()


tile.TileContext._drain_and_barrier = _patched_drain_and_barrier

_MAX_WAITS = {}
_DEFAULT_MAX_WAITS = 1
_split_ctr = [0]


def _split_excess_waits(nc):
    """Hoist sync waits beyond the per-instruction-type hardware limit onto
    standalone same-engine wait instructions inserted just before."""
    for fn in nc.m.functions:
        for bb in fn.blocks:
            il = bb.instructions
            new_list = []
            changed = False
            for inst in il:
                si = inst.sync_info
                maxw = _MAX_WAITS.get(type(inst).__name__, _DEFAULT_MAX_WAITS)
                if si is not None and len(si.on_wait) > maxw:
                    waits = list(si.on_wait)
                    extra, keep = waits[:-maxw], waits[-maxw:]
                    for j in range(0, len(extra), 2):
                        _split_ctr[0] += 1
                        w = mybir.InstEventSemaphore(
                            name=f"xw-{_split_ctr[0]}", ins=[], outs=[]
                        )
                        w.engine = inst.engine
                        w.sync_info = mybir.SyncInfo(
                            on_wait=extra[j : j + 2], on_update=[]
                        )
                        new_list.append(w)
                    si.on_wait = keep
                    changed = True
                new_list.append(inst)
            if changed:
                bb.instructions = new_list
# ---------------------------------------------------------------------------

B, S, HS, NH, NP = 2, 2048, 1024, 16, 8
HD = HS // NH  # 64
SCALE = 1.0 / np.sqrt(HD)
N_CORES = 8
GH = 4  # heads per core
DL = GH * HD  # 256 local channels
CT = HS // 128  # 8 contraction tiles
QT = 4  # q tiles of 512
KT = S // 128  # 16 k partition tiles
BF16 = mybir.dt.bfloat16
F32 = mybir.dt.float32

_cache = {}
SC512 = False


def _build(apply_mask: bool, split_waits: bool = True, knobs=()) -> bass.Bass:
    knobs = set(knobs)
    nc = bass.Bass()

    xq = nc.declare_dram_parameter("xq", [128, CT, S], BF16, isOutput=False)
    xk = nc.declare_dram_parameter("xk", [128, CT, S], BF16, isOutput=False)
    xv = nc.declare_dram_parameter("xv", [128, CT, S], BF16, isOutput=False)
    wq = nc.declare_dram_parameter("wq", [128, CT, DL], BF16, isOutput=False)
    wk = nc.declare_dram_parameter("wk", [128, CT, DL], BF16, isOutput=False)
    wv = nc.declare_dram_parameter("wv", [128, CT, DL], BF16, isOutput=False)
    wo = nc.declare_dram_parameter("wo", [128, 2, HS], BF16, isOutput=False)
    bq_c = nc.declare_dram_parameter("bq_c", [128, 2], F32, isOutput=False)
    bk_c = nc.declare_dram_parameter("bk_c", [128, 2], F32, isOutput=False)
    bv_b = nc.declare_dram_parameter("bv_b", [128, 4, 64], F32, isOutput=False)
    wp1 = nc.declare_dram_parameter("wp1", [128, CT, 512], BF16, isOutput=False)
    bp1_c = nc.declare_dram_parameter("bp1_c", [128, 4], F32, isOutput=False)
    wp2 = nc.declare_dram_parameter("wp2", [128, 4, NP], BF16, isOutput=False)
    bp2_c = nc.declare_dram_parameter("bp2_c", [NP, 1], F32, isOutput=False)
    patm = nc.declare_dram_parameter("patm", [NP, GH], BF16, isOutput=False)
    if apply_mask:
        mk = nc.declare_dram_parameter("mk", [128, KT, S], BF16, isOutput=False)

    attn_ext = nc.declare_dram_parameter("attn_t", [GH, QT, 128, KT, 512], BF16, isOutput=True)
    out_ext = nc.declare_dram_parameter("out_part", [S, HS], F32, isOutput=True)

    with tile.TileContext(nc) as tc, nc.allow_low_precision(
        reason="bf16 intermediate precision is within the 2e-2 rel-err budget"
    ):
        from contextlib import ExitStack

        with ExitStack() as outer:
            const = outer.enter_context(tc.tile_pool(name="const", bufs=1))
            psA = outer.enter_context(tc.tile_pool(name="psA", bufs=(4 if SC512 else 2), space="PSUM"))
            psB = outer.enter_context(tc.tile_pool(name="psB", bufs=2, space="PSUM"))
            psCT = outer.enter_context(tc.tile_pool(name="psCT", bufs=2, space="PSUM"))

            # ---- persistent SBUF tensors ----
            wq_sb = const.tile([128, CT, DL], BF16, tag="wq")
            wk_sb = const.tile([128, CT, DL], BF16, tag="wk")
            wv_sb = const.tile([128, CT, DL], BF16, tag="wv")
            wo_sb = const.tile([128, 2, HS], BF16, tag="wo")
            wp1_sb = const.tile([128, CT, 512], BF16, tag="wp1")
            wp2_sb = const.tile([128, 4, NP], BF16, tag="wp2")
            patm_sb = const.tile([NP, GH], BF16, tag="patm")
            bq_sb = const.tile([128, 2], F32, tag="bq")
            bk_sb = const.tile([128, 2], F32, tag="bk")
            bv_sb = const.tile([128, 4, 64], F32, tag="bv")
            bp1_sb = const.tile([128, 4], F32, tag="bp1")
            bp2_sb = const.tile([NP, 1], F32, tag="bp2")
            qt_sb = const.tile([128, 2, S], BF16, tag="qt")
            kt_sb = const.tile([128, 2, S], BF16, tag="kt")
            vaug_sb = const.tile([128, KT, GH, HD + 1], BF16, tag="vaug")
            ctn_sb = const.tile([128, 2, S], BF16, tag="ctn")
            ones128 = const.tile([1, 128], BF16, tag="ones128")
            pat_bc = const.tile([128, GH], F32, tag="patbc")

            nc.sync.dma_start(out=wv_sb[:], in_=wv[:])
            nc.sync.dma_start(out=bv_sb[:], in_=bv_b[:])
            nc.vector.memset(ones128[:], 1.0)
            nc.vector.memset(vaug_sb[:, :, :, HD : HD + 1], 1.0)

            with ExitStack() as xph:
                xpool = xph.enter_context(tc.tile_pool(name="xpool", bufs=1))
                xq_sb = xpool.tile([128, CT, S], BF16, tag="xq")
                xk_sb = xpool.tile([128, CT, S], BF16, tag="xk")
                xv_sb = xpool.tile([128, CT, S], BF16, tag="xv")
                for ct in range(CT):
                    nc.sync.dma_start(out=xv_sb[:, ct, :], in_=xv[:, ct, :])
                nc.sync.dma_start(out=wq_sb[:], in_=wq[:])
                nc.sync.dma_start(out=bq_sb[:], in_=bq_c[:])
                for ct in range(CT):
                    nc.sync.dma_start(out=xq_sb[:, ct, :], in_=xq[:, ct, :])
                nc.sync.dma_start(out=wk_sb[:], in_=wk[:])
                nc.sync.dma_start(out=bk_sb[:], in_=bk_c[:])
                for ct in range(CT):
                    nc.sync.dma_start(out=xk_sb[:, ct, :], in_=xk[:, ct, :])
                nc.sync.dma_start(out=wo_sb[:], in_=wo[:])
                nc.sync.dma_start(out=wp1_sb[:], in_=wp1[:])
                nc.sync.dma_start(out=wp2_sb[:], in_=wp2[:])
                nc.sync.dma_start(out=patm_sb[:], in_=patm[:])
                nc.sync.dma_start(out=bp1_sb[:], in_=bp1_c[:])
                nc.sync.dma_start(out=bp2_sb[:], in_=bp2_c[:])

                sm = xph.enter_context(tc.tile_pool(name="sel", bufs=1))

                # ---- projections ----
                # V first (feeds attn@V); out[tok,128..] = x^T tile as lhsT
                for tk in range(KT):
                    v_ps = psB.tile([128, DL], F32, tag="b")
                    for ct in range(CT):
                        nc.tensor.matmul(
                            out=v_ps[:],
                            lhsT=xv_sb[:, ct, tk * 128 : (tk + 1) * 128],
                            rhs=wv_sb[:, ct, :],
                            start=(ct == 0),
                            stop=(ct == CT - 1),
                        )
                    nc.vector.tensor_tensor(
                        out=vaug_sb[:, tk, :, 0:HD],
                        in0=v_ps[:].rearrange("p (h d) -> p h d", h=GH),
                        in1=bv_sb[:],
                        op=mybir.AluOpType.add,
                    )

                # ---- pattern selector (tiny) ----
                mq_f = sm.tile([128, CT], F32, tag="mqf")
                for ct in range(CT):
                    nc.vector.tensor_reduce(
                        out=mq_f[:, ct : ct + 1],
                        in_=xq_sb[:, ct, :],
                        axis=mybir.AxisListType.X,
                        op=mybir.AluOpType.add,
                    )
                mq_bf = sm.tile([128, CT], BF16, tag="mqb")
                nc.vector.tensor_copy(out=mq_bf[:], in_=mq_f[:])

                h_sb = sm.tile([128, 4], BF16, tag="hsb")
                for jt in range(4):
                    h_ps = psB.tile([128, 1], F32, tag="b")
                    for ct in range(CT):
                        nc.tensor.matmul(
                            out=h_ps[:],
                            lhsT=wp1_sb[:, ct, jt * 128 : (jt + 1) * 128],
                            rhs=mq_bf[:, ct : ct + 1],
                            start=(ct == 0),
                            stop=(ct == CT - 1),
                        )
                    nc.scalar.activation(
                        out=h_sb[:, jt : jt + 1],
                        in_=h_ps[:],
                        func=mybir.ActivationFunctionType.Relu,
                        bias=bp1_sb[:, jt : jt + 1],
                    )

                lg_ps = psB.tile([NP, 1], F32, tag="b")
                for jt in range(4):
                    nc.tensor.matmul(
                        out=lg_ps[:],
                        lhsT=wp2_sb[:, jt, :],
                        rhs=h_sb[:, jt : jt + 1],
                        start=(jt == 0),
                        stop=(jt == 3),
                    )
                explg = sm.tile([NP, 1], BF16, tag="explg")
                nc.scalar.activation(
                    out=explg[:],
                    in_=lg_ps[:],
                    func=mybir.ActivationFunctionType.Exp,
                    bias=bp2_sb[:],
                )
                ones_np = sm.tile([NP, 1], BF16, tag="onesnp")
                nc.vector.memset(ones_np[:], 1.0)
                den_ps = psB.tile([1, 1], F32, tag="b")
                nc.tensor.matmul(
                    out=den_ps[:], lhsT=ones_np[:],
                    rhs=explg[:], start=True, stop=True,
                )
                rec_f = sm.tile([1, 1], F32, tag="recf")
                nc.vector.reciprocal(out=rec_f[:], in_=den_ps[:])
                rec_b = sm.tile([1, 1], BF16, tag="recb")
                nc.vector.tensor_copy(out=rec_b[:], in_=rec_f[:])
                bc8_ps = psB.tile([NP, 1], F32, tag="b")
                nc.tensor.matmul(
                    out=bc8_ps[:], lhsT=ones128[:, 0:NP], rhs=rec_b[:],
                    start=True, stop=True,
                )
                pw = sm.tile([NP, 1], BF16, tag="pw")
                nc.vector.tensor_tensor(
                    out=pw[:], in0=explg[:], in1=bc8_ps[:],
                    op=mybir.AluOpType.mult,
                )
                pat4_ps = psB.tile([1, GH], F32, tag="b")
                nc.tensor.matmul(
                    out=pat4_ps[:], lhsT=pw[:], rhs=patm_sb[:],
                    start=True, stop=True,
                )
                pat4_b = sm.tile([1, GH], BF16, tag="pat4")
                nc.vector.tensor_copy(out=pat4_b[:], in_=pat4_ps[:])
                for h in range(GH):
                    pbc_ps = psB.tile([128, 1], F32, tag="b")
                    nc.tensor.matmul(
                        out=pbc_ps[:], lhsT=ones128[:], rhs=pat4_b[:, h : h + 1],
                        start=True, stop=True,
                    )
                    nc.vector.tensor_copy(out=pat_bc[:, h : h + 1], in_=pbc_ps[:])

                # Q^T, K^T: [dlocal, tokens]
                for pt in range(2):
                    for tt in range(QT):
                        q_ps = psA.tile([128, 512], F32, tag="a")
                        for ct in range(CT):
                            nc.tensor.matmul(
                                out=q_ps[:],
                                lhsT=wq_sb[:, ct, pt * 128 : (pt + 1) * 128],
                                rhs=xq_sb[:, ct, tt * 512 : (tt + 1) * 512],
                                start=(ct == 0),
                                stop=(ct == CT - 1),
                            )
                        nc.vector.tensor_scalar(
                            out=qt_sb[:, pt, tt * 512 : (tt + 1) * 512],
                            in0=q_ps[:],
                            scalar1=bq_sb[:, pt : pt + 1],
                            scalar2=None,
                            op0=mybir.AluOpType.add,
                        )
                        k_ps = psA.tile([128, 512], F32, tag="a")
                        for ct in range(CT):
                            nc.tensor.matmul(
                                out=k_ps[:],
                                lhsT=wk_sb[:, ct, pt * 128 : (pt + 1) * 128],
                                rhs=xk_sb[:, ct, tt * 512 : (tt + 1) * 512],
                                start=(ct == 0),
                                stop=(ct == CT - 1),
                            )
                        nc.vector.tensor_scalar(
                            out=kt_sb[:, pt, tt * 512 : (tt + 1) * 512],
                            in0=k_ps[:],
                            scalar1=bk_sb[:, pt : pt + 1],
                            scalar2=None,
                            op0=mybir.AluOpType.add,
                        )

            # ---- attention ----
            with ExitStack() as aph:
                if apply_mask:
                    mpool = aph.enter_context(tc.tile_pool(name="mpool", bufs=1))
                    mk_sb = mpool.tile([128, KT, S], BF16, tag="mk")
                    nc.sync.dma_start(out=mk_sb[:], in_=mk[:])
                epool = aph.enter_context(tc.tile_pool(name="epool", bufs=3))
                stpool = aph.enter_context(tc.tile_pool(name="stpool", bufs=3))
                bcpool = aph.enter_context(tc.tile_pool(name="bcpool", bufs=3))
                opool = aph.enter_context(tc.tile_pool(name="opool", bufs=2))

                for qt in range(QT):
                    for hp in range(2):  # head pairs: rows 0-63 / 64-127
                        exps = []
                        cts = []
                        for hh in range(2):
                            exp_sb = epool.tile([128, KT, 512], BF16, tag="exp")
                            ct_ps = psCT.tile([HD + 1, 512], F32, tag="ct")
                            exps.append(exp_sb)
                            cts.append(ct_ps)
                        for kj in range(KT // 2):
                            for hh in range(2):
                                h = 2 * hp + hh
                                r0 = hh * HD
                                if SC512:
                                    for half in range(2 if "no_scores" not in knobs else 0):
                                        kt = 2 * kj + half
                                        sc_ps = psA.tile([128, 512], F32, tag="a")
                                        nc.tensor.matmul(
                                            out=sc_ps[:],
                                            lhsT=kt_sb[
                                                r0 : r0 + HD, hp, kt * 128 : (kt + 1) * 128
                                            ],
                                            rhs=qt_sb[
                                                r0 : r0 + HD, hp,
                                                qt * 512 : (qt + 1) * 512,
                                            ],
                                            start=True,
                                            stop=True,
                                        )
                                        nc.scalar.activation(
                                            out=exps[hh][:, kt, :],
                                            in_=sc_ps[:],
                                            func=mybir.ActivationFunctionType.Exp,
                                            scale=pat_bc[:, h : h + 1],
                                        )
                                else:
                                    sc_ps = psA.tile([128, 1024], F32, tag="a")
                                    for half in range(2 if "no_scores" not in knobs else 0):
                                        kt = 2 * kj + half
                                        nc.tensor.matmul(
                                            out=sc_ps[:, half * 512 : (half + 1) * 512],
                                            lhsT=kt_sb[
                                                r0 : r0 + HD, hp, kt * 128 : (kt + 1) * 128
                                            ],
                                            rhs=qt_sb[
                                                r0 : r0 + HD, hp,
                                                qt * 512 : (qt + 1) * 512,
                                            ],
                                            start=True,
                                            stop=True,
                                        )
                                    if "no_exp" in knobs:
                                        nc.vector.memset(exps[hh][:, 2 * kj : 2 * kj + 2, :], 0.5)
                                    else:
                                        nc.scalar.activation(
                                            out=exps[hh][:, 2 * kj : 2 * kj + 2, :],
                                            in_=sc_ps[:].rearrange(
                                                "p (a b) -> p a b", a=2
                                            ),
                                            func=mybir.ActivationFunctionType.Exp,
                                            scale=pat_bc[:, h : h + 1],
                                        )
                                if apply_mask:
                                    nc.vector.tensor_tensor(
                                        out=exps[hh][:, 2 * kj : 2 * kj + 2, :],
                                        in0=exps[hh][:, 2 * kj : 2 * kj + 2, :],
                                        in1=mk_sb[
                                            :, 2 * kj : 2 * kj + 2,
                                            qt * 512 : (qt + 1) * 512,
                                        ],
                                        op=mybir.AluOpType.mult,
                                    )
                                for half in range(2 if "no_av" not in knobs else 0):
                                    kt = 2 * kj + half
                                    nc.tensor.matmul(
                                        out=cts[hh][:],
                                        lhsT=vaug_sb[:, kt, 2 * hp + hh, :],
                                        rhs=exps[hh][:, kt, :],
                                        start=(kt == 0),
                                        stop=(kt == KT - 1),
                                    )
                        bcs = []
                        for hh in range(2):
                            h = 2 * hp + hh
                            ct_ps = cts[hh]
                            if "no_av" in knobs:
                                nc.vector.memset(ct_ps[:], 1.0)
                            sum_b = bcpool.tile([1, 512], BF16, tag="sumb")
                            nc.vector.tensor_copy(out=sum_b[:], in_=ct_ps[HD : HD + 1, :])
                            bc_ps = psB.tile([128, 512], F32, tag="b")
                            nc.tensor.matmul(
                                out=bc_ps[:], lhsT=ones128[:], rhs=sum_b[:],
                                start=True, stop=True,
                            )
                            bc_f = bcpool.tile([128, 512], F32, tag="bcf")
                            nc.vector.reciprocal(out=bc_f[:], in_=bc_ps[:])
                            bc_bf = bcpool.tile([128, 512], BF16, tag="bcbf")
                            nc.vector.tensor_copy(out=bc_bf[:], in_=bc_f[:])
                            # context normalize -> ctn[dlocal, tokens]
                            nc.vector.tensor_tensor(
                                out=ctn_sb[
                                    hh * HD : (hh + 1) * HD, hp,
                                    qt * 512 : (qt + 1) * 512,
                                ],
                                in0=ct_ps[0:HD, :],
                                in1=bc_bf[0:HD, :],
                                op=mybir.AluOpType.mult,
                            )
                            bcs.append(bc_bf)
                        for hh in range(2):
                            h = 2 * hp + hh
                            exp_sb = exps[hh]
                            bc_bf = bcs[hh]
                            # attn normalize + stage + DMA out
                            attn_st = stpool.tile([128, KT, 512], BF16, tag="st")
                            if "no_norm" not in knobs:
                                for kt in range(KT):
                                    nc.vector.tensor_tensor(
                                        out=attn_st[:, kt, :],
                                        in0=exp_sb[:, kt, :],
                                        in1=bc_bf[:],
                                        op=mybir.AluOpType.mult,
                                    )
                            else:
                                nc.vector.tensor_copy(out=attn_st[:, 0, :], in_=exp_sb[:, 0, :])
                            if "no_attn_dma" not in knobs and "no_norm" not in knobs:
                                nc.gpsimd.dma_start(
                                    out=attn_ext[h, qt], in_=attn_st[:]
                                )

                    # ---- output projection, delayed one qt for overlap ----
                    for tt4 in range(4 if qt > 0 else 0):
                        tt = (qt - 1) * 4 + tt4
                        ostage = opool.tile([128, HS], F32, tag="ost")
                        for oh in range(2):
                            o_ps = psB.tile([128, 512], F32, tag="b")
                            for pt in range(2):
                                nc.tensor.matmul(
                                    out=o_ps[:],
                                    lhsT=ctn_sb[:, pt, tt * 128 : (tt + 1) * 128],
                                    rhs=wo_sb[:, pt, oh * 512 : (oh + 1) * 512],
                                    start=(pt == 0),
                                    stop=(pt == 1),
                                )
                            nc.scalar.copy(
                                out=ostage[:, oh * 512 : (oh + 1) * 512], in_=o_ps[:]
                            )
                        nc.gpsimd.dma_start(
                            out=out_ext[tt * 128 : (tt + 1) * 128, :], in_=ostage[:]
                        )

                for tt4 in range(4):
                    tt = (QT - 1) * 4 + tt4
                    ostage = opool.tile([128, HS], F32, tag="ost")
                    for oh in range(2):
                        o_ps = psB.tile([128, 512], F32, tag="b")
                        for pt in range(2):
                            nc.tensor.matmul(
                                out=o_ps[:],
                                lhsT=ctn_sb[:, pt, tt * 128 : (tt + 1) * 128],
                                rhs=wo_sb[:, pt, oh * 512 : (oh + 1) * 512],
                                start=(pt == 0),
                                stop=(pt == 1),
                            )
                        nc.scalar.copy(
                            out=ostage[:, oh * 512 : (oh + 1) * 512], in_=o_ps[:]
                        )
                    nc.gpsimd.dma_start(
                        out=out_ext[tt * 128 : (tt + 1) * 128, :], in_=ostage[:]
                    )

    if split_waits:
        _split_excess_waits(nc)
    return nc


def _prep_inputs(query, key, value, attention_mask, Wq, bq, Wk, bk, Wv, bv,
                 Wo, bo, Wp1, bp1, Wp2, bp2, patterns, apply_mask):
    bf = ml_dtypes.bfloat16
    f32 = np.float32

    def arr_x(x_b):  # [S, HS] -> [128, CT, S] bf16 of x_b.T
        t = np.ascontiguousarray(x_b.T.reshape(CT, 128, S).transpose(1, 0, 2))
        return t.astype(bf)

    def arr_w(W, g):  # [HS, HS] -> [128, CT, DL] col slice
        Wg = W[:, g * DL : (g + 1) * DL]
        return np.ascontiguousarray(
            Wg.reshape(CT, 128, DL).transpose(1, 0, 2)
        ).astype(bf)

    xqs = [arr_x(np.asarray(query[b], f32)) for b in range(B)]
    xks = [arr_x(np.asarray(key[b], f32)) for b in range(B)]
    xvs = [arr_x(np.asarray(value[b], f32)) for b in range(B)]
    wp1_a = np.ascontiguousarray(
        (np.asarray(Wp1, f32) / S).reshape(CT, 128, 512).transpose(1, 0, 2)
    ).astype(bf)
    bp1_a = np.ascontiguousarray(np.asarray(bp1, f32).reshape(4, 128).T)
    wp2_a = np.ascontiguousarray(
        np.asarray(Wp2, f32).reshape(4, 128, NP).transpose(1, 0, 2)
    ).astype(bf)
    bp2_a = np.asarray(bp2, f32).reshape(NP, 1).copy()
    patterns_r = np.asarray(patterns, f32).reshape(NP, NH)

    if apply_mask:
        mks = []
        for b in range(B):
            mt = np.asarray(attention_mask[b], f32).T  # [k, q]
            mks.append(
                np.ascontiguousarray(
                    mt.reshape(KT, 128, S).transpose(1, 0, 2)
                ).astype(bf)
            )

    in_maps = []
    for c in range(N_CORES):
        b, g = c // 4, c % 4
        im = {
            "xq": xqs[b], "xk": xks[b], "xv": xvs[b],
            "wq": arr_w(np.asarray(Wq, f32), g),
            "wk": arr_w(np.asarray(Wk, f32), g),
            "wv": arr_w(np.asarray(Wv, f32), g),
            "wo": np.ascontiguousarray(
                np.asarray(Wo, f32)[g * DL : (g + 1) * DL, :]
                .reshape(2, 128, HS).transpose(1, 0, 2)
            ).astype(bf),
            "bq_c": np.ascontiguousarray(
                np.asarray(bq, f32)[g * DL : (g + 1) * DL].reshape(2, 128).T
            ),
            "bk_c": np.ascontiguousarray(
                np.asarray(bk, f32)[g * DL : (g + 1) * DL].reshape(2, 128).T
            ),
            "bv_b": np.ascontiguousarray(
                np.broadcast_to(
                    np.asarray(bv, f32)[g * DL : (g + 1) * DL].reshape(1, 4, 64),
                    (128, 4, 64),
                )
            ),
            "wp1": wp1_a, "bp1_c": bp1_a, "wp2": wp2_a, "bp2_c": bp2_a,
            "patm": np.ascontiguousarray(
                patterns_r[:, g * GH : (g + 1) * GH] * SCALE
            ).astype(bf),
        }
        if apply_mask:
            im["mk"] = mks[b]
        in_maps.append(im)
    return in_maps


def kernel(**inputs):
    attention_mask = np.asarray(inputs["attention_mask"])
    apply_mask = not bool(attention_mask.all())

    if apply_mask not in _cache:
        _cache[apply_mask] = _build(apply_mask)
    nc = _cache[apply_mask]

    in_maps = _prep_inputs(
        inputs["query"], inputs["key"], inputs["value"], attention_mask,
        inputs["Wq"], inputs["bq"], inputs["Wk"], inputs["bk"],
        inputs["Wv"], inputs["bv"], inputs["Wo"], inputs["bo"],
        inputs["Wp1"], inputs["bp1"], inputs["Wp2"], inputs["bp2"],
        inputs["patterns"], apply_mask,
    )

    res = run_bass_kernel_spmd(nc, in_maps, core_ids=list(range(N_CORES)))

    f32 = np.float32
    output = np.zeros((B, S, HS), f32)
    attn = np.empty((B, NH, S, S), f32)
    for c in range(N_CORES):
        b, g = c // 4, c % 4
        r = res.results[c]
        output[b] += np.asarray(r["out_part"], f32)
        at = np.asarray(r["attn_t"])  # [GH, QT, 128(p), KT, 512(ql)]
        attn[b, g * GH : (g + 1) * GH] = (
            at.transpose(0, 1, 4, 3, 2).reshape(GH, S, S).astype(f32)
        )
    output += np.asarray(inputs["bo"], f32)[None, None, :]
    return output, attn


# revision 29
# speedup vs baseline: 32.3590x; 1.1844x over previous
"""AdaptiveAttention Trainium2 kernel (8 NeuronCores, SPMD, no collectives).

Sharding: core c -> batch b=c//4, head group g=c%4 (4 of 16 heads).
Each core computes its 4 heads' Q/K/V projections (256 of 1024 channels)
for its batch, full attention for those heads in transposed layout
(scoresT[k,q] = K Q^T so softmax sums come free from a ones-row fused
into the attn@V matmul), writes attn_weights^T in bf16 plus a partial
output projection. Host sums the 4 partials per batch, adds bo, and
transposes attn weights back.
"""

import os
import sys

for _p in ("/opt/trn_rl_repo", "/root/.axon_site/_ro/trn_rl_repo"):
    if os.path.isdir(_p) and _p not in sys.path:
        sys.path.insert(0, _p)

import ml_dtypes
import numpy as np

import concourse.bass as bass
import concourse.tile as tile
from concourse import mybir
from concourse.bass_utils import run_bass_kernel_spmd

# ---------------------------------------------------------------------------
# Workaround: walrus rejects the TileContext tail drain when it carries >2
# sem waits ("Too many sync wait commands"). Emit standalone 1-wait SP
# instructions instead.
from bass_rust import ScopedClock

_WAIT_MODE = {"sem-ge-imm": "sem-ge", "sem-eq-imm": "sem-eq"}


def _patched_drain_and_barrier(self, tick_clock, wait_clock):
    nc = self.nc
    drain_inst = nc.sync.drain()
    wait_clock.add_sem_waits(
        drain_inst.ins, ScopedClock({None: tick_clock.global_clock})
    )
    si = drain_inst.ins.sync_info
    if si is not None and len(si.on_wait) > 1:
        waits = list(si.on_wait)
        si.on_wait = []
        by_num = {h.num: h for h in self.sems.allocated().values()}
        for w in waits:
            h = by_num.get(w.id)
            assert h is not None, f"no handle for sem {w.id} ({w.ant_name})"
            nc.sync.wait_op(h, w.wait_value, _WAIT_MODE[w.wait_mode])
    nc.all_engine_barrier()
    popped = nc._tile_sem_poison_stack.pop()
    assert popped is self._sem_poison
    nc.clear_and_free_semaphores(list(self.sems.allocated().values()))
    nc.all_engine_barrier()


tile.TileContext._drain_and_barrier = _patched_drain_and_barrier

_MAX_WAITS = {}
_DEFAULT_MAX_WAITS = 1
_split_ctr = [0]


def _split_excess_waits(nc):
    """Hoist sync waits beyond the per-instruction-type hardware limit onto
    standalone same-engine wait instructions inserted just before."""
    for fn in nc.m.functions:
        for bb in fn.blocks:
            il = bb.instructions
            new_list = []
            changed = False
            for inst in il:
                si = inst.sync_info
                maxw = _MAX_WAITS.get(type(inst).__name__, _DEFAULT_MAX_WAITS)
                if si is not None and len(si.on_wait) > maxw:
                    waits = list(si.on_wait)
                    extra, keep = waits[:-maxw], waits[-maxw:]
                    for j in range(0, len(extra), 2):
                        _split_ctr[0] += 1
                        w = mybir.InstEventSemaphore(
                            name=f"xw-{_split_ctr[0]}", ins=[], outs=[]
                        )
                        w.engine = inst.engine
                        w.sync_info = mybir.SyncInfo(
                            on_wait=extra[j : j + 2], on_update=[]
                        )
                        new_list.append(w)
                    si.on_wait = keep
                    changed = True
                new_list.append(inst)
            if changed:
                bb.instructions = new_list
# ---------------------------------------------------------------------------

B, S, HS, NH, NP = 2, 2048, 1024, 16, 8
HD = HS // NH  # 64
SCALE = 1.0 / np.sqrt(HD)
N_CORES = 8
GH = 4  # heads per core
DL = GH * HD  # 256 local channels
CT = HS // 128  # 8 contraction tiles
QT = 4  # q tiles of 512
KT = S // 128  # 16 k partition tiles
BF16 = mybir.dt.bfloat16
F32 = mybir.dt.float32

_cache = {}
SC512 = False


def _build(apply_mask: bool, split_waits: bool = True, knobs=()) -> bass.Bass:
    knobs = set(knobs)
    nc = bass.Bass()

    xq = nc.declare_dram_parameter("xq", [128, CT, S], BF16, isOutput=False)
    xk = nc.declare_dram_parameter("xk", [128, CT, S], BF16, isOutput=False)
    xv = nc.declare_dram_parameter("xv", [128, CT, S], BF16, isOutput=False)
    wq = nc.declare_dram_parameter("wq", [128, CT, DL], BF16, isOutput=False)
    wk = nc.declare_dram_parameter("wk", [128, CT, DL], BF16, isOutput=False)
    wv = nc.declare_dram_parameter("wv", [128, CT, DL], BF16, isOutput=False)
    wo = nc.declare_dram_parameter("wo", [128, 2, HS], BF16, isOutput=False)
    bq_c = nc.declare_dram_parameter("bq_c", [128, 2], F32, isOutput=False)
    bk_c = nc.declare_dram_parameter("bk_c", [128, 2], F32, isOutput=False)
    bv_b = nc.declare_dram_parameter("bv_b", [128, 4, 64], F32, isOutput=False)
    wp1 = nc.declare_dram_parameter("wp1", [128, CT, 512], BF16, isOutput=False)
    bp1_c = nc.declare_dram_parameter("bp1_c", [128, 4], F32, isOutput=False)
    wp2 = nc.declare_dram_parameter("wp2", [128, 4, NP], BF16, isOutput=False)
    bp2_c = nc.declare_dram_parameter("bp2_c", [NP, 1], F32, isOutput=False)
    patm = nc.declare_dram_parameter("patm", [NP, GH], BF16, isOutput=False)
    if apply_mask:
        mk = nc.declare_dram_parameter("mk", [128, KT, S], BF16, isOutput=False)

    attn_ext = nc.declare_dram_parameter("attn_t", [GH, QT, 128, KT, 512], BF16, isOutput=True)
    out_ext = nc.declare_dram_parameter("out_part", [S, HS], F32, isOutput=True)

    with tile.TileContext(nc) as tc, nc.allow_low_precision(
        reason="bf16 intermediate precision is within the 2e-2 rel-err budget"
    ):
        from contextlib import ExitStack

        with ExitStack() as outer:
            const = outer.enter_context(tc.tile_pool(name="const", bufs=1))
            psA = outer.enter_context(tc.tile_pool(name="psA", bufs=(4 if SC512 else 2), space="PSUM"))
            psB = outer.enter_context(tc.tile_pool(name="psB", bufs=2, space="PSUM"))
            psCT = outer.enter_context(tc.tile_pool(name="psCT", bufs=2, space="PSUM"))

            # ---- persistent SBUF tensors ----
            wq_sb = const.tile([128, CT, DL], BF16, tag="wq")
            wk_sb = const.tile([128, CT, DL], BF16, tag="wk")
            wv_sb = const.tile([128, CT, DL], BF16, tag="wv")
            wo_sb = const.tile([128, 2, HS], BF16, tag="wo")
            wp1_sb = const.tile([128, CT, 512], BF16, tag="wp1")
            wp2_sb = const.tile([128, 4, NP], BF16, tag="wp2")
            patm_sb = const.tile([NP, GH], BF16, tag="patm")
            bq_sb = const.tile([128, 2], F32, tag="bq")
            bk_sb = const.tile([128, 2], F32, tag="bk")
            bv_sb = const.tile([128, 4, 64], F32, tag="bv")
            bp1_sb = const.tile([128, 4], F32, tag="bp1")
            bp2_sb = const.tile([NP, 1], F32, tag="bp2")
            qt_sb = const.tile([128, 2, S], BF16, tag="qt")
            kt_sb = const.tile([128, 2, S], BF16, tag="kt")
            vaug_sb = const.tile([128, KT, GH, HD + 1], BF16, tag="vaug")
            ctn_sb = const.tile([128, 2, S], BF16, tag="ctn")
            ones128 = const.tile([1, 128], BF16, tag="ones128")
            pat_bc = const.tile([128, GH], F32, tag="patbc")

            nc.sync.dma_start(out=wv_sb[:], in_=wv[:])
            nc.sync.dma_start(out=bv_sb[:], in_=bv_b[:])
            nc.vector.memset(ones128[:], 1.0)
            nc.vector.memset(vaug_sb[:, :, :, HD : HD + 1], 1.0)

            with ExitStack() as xph:
                xpool = xph.enter_context(tc.tile_pool(name="xpool", bufs=1))
                xq_sb = xpool.tile([128, CT, S], BF16, tag="xq")
                xk_sb = xpool.tile([128, CT, S], BF16, tag="xk")
                xv_sb = xpool.tile([128, CT, S], BF16, tag="xv")
                for ct in range(CT):
                    nc.sync.dma_start(out=xv_sb[:, ct, :], in_=xv[:, ct, :])
                nc.sync.dma_start(out=wq_sb[:], in_=wq[:])
                nc.sync.dma_start(out=bq_sb[:], in_=bq_c[:])
                for ct in range(CT):
                    nc.sync.dma_start(out=xq_sb[:, ct, :], in_=xq[:, ct, :])
                nc.sync.dma_start(out=wk_sb[:], in_=wk[:])
                nc.sync.dma_start(out=bk_sb[:], in_=bk_c[:])
                for ct in range(CT):
                    nc.sync.dma_start(out=xk_sb[:, ct, :], in_=xk[:, ct, :])
                nc.sync.dma_start(out=wo_sb[:], in_=wo[:])
                nc.sync.dma_start(out=wp1_sb[:], in_=wp1[:])
                nc.sync.dma_start(out=wp2_sb[:], in_=wp2[:])
                nc.sync.dma_start(out=patm_sb[:], in_=patm[:])
                nc.sync.dma_start(out=bp1_sb[:], in_=bp1_c[:])
                nc.sync.dma_start(out=bp2_sb[:], in_=bp2_c[:])

                sm = xph.enter_context(tc.tile_pool(name="sel", bufs=1))

                # ---- projections ----
                # V first (feeds attn@V); out[tok,128..] = x^T tile as lhsT
                for tk in range(KT):
                    v_ps = psB.tile([128, DL], F32, tag="b")
                    for ct in range(CT):
                        nc.tensor.matmul(
                            out=v_ps[:],
                            lhsT=xv_sb[:, ct, tk * 128 : (tk + 1) * 128],
                            rhs=wv_sb[:, ct, :],
                            start=(ct == 0),
                            stop=(ct == CT - 1),
                        )
                    nc.vector.tensor_tensor(
                        out=vaug_sb[:, tk, :, 0:HD],
                        in0=v_ps[:].rearrange("p (h d) -> p h d", h=GH),
                        in1=bv_sb[:],
                        op=mybir.AluOpType.add,
                    )

                # ---- pattern selector (tiny) ----
                mq_f = sm.tile([128, CT], F32, tag="mqf")
                for ct in range(CT):
                    nc.vector.tensor_reduce(
                        out=mq_f[:, ct : ct + 1],
                        in_=xq_sb[:, ct, :],
                        axis=mybir.AxisListType.X,
                        op=mybir.AluOpType.add,
                    )
                mq_bf = sm.tile([128, CT], BF16, tag="mqb")
                nc.vector.tensor_copy(out=mq_bf[:], in_=mq_f[:])

                h_sb = sm.tile([128, 4], BF16, tag="hsb")
                for jt in range(4):
                    h_ps = psB.tile([128, 1], F32, tag="b")
                    for ct in range(CT):
                        nc.tensor.matmul(
                            out=h_ps[:],
                            lhsT=wp1_sb[:, ct, jt * 128 : (jt + 1) * 128],
                            rhs=mq_bf[:, ct : ct + 1],
                            start=(ct == 0),
                            stop=(ct == CT - 1),
                        )
                    nc.scalar.activation(
                        out=h_sb[:, jt : jt + 1],
                        in_=h_ps[:],
                        func=mybir.ActivationFunctionType.Relu,
                        bias=bp1_sb[:, jt : jt + 1],
                    )

                lg_ps = psB.tile([NP, 1], F32, tag="b")
                for jt in range(4):
                    nc.tensor.matmul(
                        out=lg_ps[:],
                        lhsT=wp2_sb[:, jt, :],
                        rhs=h_sb[:, jt : jt + 1],
                        start=(jt == 0),
                        stop=(jt == 3),
                    )
                explg = sm.tile([NP, 1], BF16, tag="explg")
                nc.scalar.activation(
                    out=explg[:],
                    in_=lg_ps[:],
                    func=mybir.ActivationFunctionType.Exp,
                    bias=bp2_sb[:],
                )
                ones_np = sm.tile([NP, 1], BF16, tag="onesnp")
                nc.vector.memset(ones_np[:], 1.0)
                den_ps = psB.tile([1, 1], F32, tag="b")
                nc.tensor.matmul(
                    out=den_ps[:], lhsT=ones_np[:],
                    rhs=explg[:], start=True, stop=True,
                )
                rec_f = sm.tile([1, 1], F32, tag="recf")
                nc.vector.reciprocal(out=rec_f[:], in_=den_ps[:])
                rec_b = sm.tile([1, 1], BF16, tag="recb")
                nc.vector.tensor_copy(out=rec_b[:], in_=rec_f[:])
                bc8_ps = psB.tile([NP, 1], F32, tag="b")
                nc.tensor.matmul(
                    out=bc8_ps[:], lhsT=ones128[:, 0:NP], rhs=rec_b[:],
                    start=True, stop=True,
                )
                pw = sm.tile([NP, 1], BF16, tag="pw")
                nc.vector.tensor_tensor(
                    out=pw[:], in0=explg[:], in1=bc8_ps[:],
                    op=mybir.AluOpType.mult,
                )
                pat4_ps = psB.tile([1, GH], F32, tag="b")
                nc.tensor.matmul(
                    out=pat4_ps[:], lhsT=pw[:], rhs=patm_sb[:],
                    start=True, stop=True,
                )
                pat4_b = sm.tile([1, GH], BF16, tag="pat4")
                nc.vector.tensor_copy(out=pat4_b[:], in_=pat4_ps[:])
                for h in range(GH):
                    pbc_ps = psB.tile([128, 1], F32, tag="b")
                    nc.tensor.matmul(
                        out=pbc_ps[:], lhsT=ones128[:], rhs=pat4_b[:, h : h + 1],
                        start=True, stop=True,
                    )
                    nc.vector.tensor_copy(out=pat_bc[:, h : h + 1], in_=pbc_ps[:])

                # Q^T, K^T: [dlocal, tokens]
                for pt in range(2):
                    for tt in range(QT):
                        q_ps = psA.tile([128, 512], F32, tag="a")
                        for ct in range(CT):
                            nc.tensor.matmul(
                                out=q_ps[:],
                                lhsT=wq_sb[:, ct, pt * 128 : (pt + 1) * 128],
                                rhs=xq_sb[:, ct, tt * 512 : (tt + 1) * 512],
                                start=(ct == 0),
                                stop=(ct == CT - 1),
                            )
                        nc.vector.tensor_scalar(
                            out=qt_sb[:, pt, tt * 512 : (tt + 1) * 512],
                            in0=q_ps[:],
                            scalar1=bq_sb[:, pt : pt + 1],
                            scalar2=None,
                            op0=mybir.AluOpType.add,
                        )
                        k_ps = psA.tile([128, 512], F32, tag="a")
                        for ct in range(CT):
                            nc.tensor.matmul(
                                out=k_ps[:],
                                lhsT=wk_sb[:, ct, pt * 128 : (pt + 1) * 128],
                                rhs=xk_sb[:, ct, tt * 512 : (tt + 1) * 512],
                                start=(ct == 0),
                                stop=(ct == CT - 1),
                            )
                        nc.vector.tensor_scalar(
                            out=kt_sb[:, pt, tt * 512 : (tt + 1) * 512],
                            in0=k_ps[:],
                            scalar1=bk_sb[:, pt : pt + 1],
                            scalar2=None,
                            op0=mybir.AluOpType.add,
                        )

            # ---- attention ----
            with ExitStack() as aph:
                if apply_mask:
                    mpool = aph.enter_context(tc.tile_pool(name="mpool", bufs=1))
                    mk_sb = mpool.tile([128, KT, S], BF16, tag="mk")
                    nc.sync.dma_start(out=mk_sb[:], in_=mk[:])
                epool = aph.enter_context(tc.tile_pool(name="epool", bufs=4))
                stpool = aph.enter_context(tc.tile_pool(name="stpool", bufs=3))
                bcpool = aph.enter_context(tc.tile_pool(name="bcpool", bufs=3))
                opool = aph.enter_context(tc.tile_pool(name="opool", bufs=2))

                from contextlib import nullcontext

                for qt in range(QT):
                  with nullcontext():
                    for hp in range(2):  # head pairs: rows 0-63 / 64-127
                        exps = []
                        cts = []
                        for hh in range(2):
                            exp_sb = epool.tile([128, KT, 512], BF16, tag="exp")
                            ct_ps = psCT.tile([HD + 1, 512], F32, tag="ct")
                            exps.append(exp_sb)
                            cts.append(ct_ps)
                        for kj in range(KT // 2):
                            for hh in range(2):
                                h = 2 * hp + hh
                                r0 = hh * HD
                                if SC512:
                                    for half in range(2 if "no_scores" not in knobs else 0):
                                        kt = 2 * kj + half
                                        sc_ps = psA.tile([128, 512], F32, tag="a")
                                        nc.tensor.matmul(
                                            out=sc_ps[:],
                                            lhsT=kt_sb[
                                                r0 : r0 + HD, hp, kt * 128 : (kt + 1) * 128
                                            ],
                                            rhs=qt_sb[
                                                r0 : r0 + HD, hp,
                                                qt * 512 : (qt + 1) * 512,
                                            ],
                                            start=True,
                                            stop=True,
                                        )
                                        nc.scalar.activation(
                                            out=exps[hh][:, kt, :],
                                            in_=sc_ps[:],
                                            func=mybir.ActivationFunctionType.Exp,
                                            scale=pat_bc[:, h : h + 1],
                                        )
                                else:
                                    sc_ps = psA.tile([128, 1024], F32, tag="a")
                                    for half in range(2 if "no_scores" not in knobs else 0):
                                        kt = 2 * kj + half
                                        nc.tensor.matmul(
                                            out=sc_ps[:, half * 512 : (half + 1) * 512],
                                            lhsT=kt_sb[
                                                r0 : r0 + HD, hp, kt * 128 : (kt + 1) * 128
                                            ],
                                            rhs=qt_sb[
                                                r0 : r0 + HD, hp,
                                                qt * 512 : (qt + 1) * 512,
                                            ],
                                            start=True,
                                            stop=True,
                                        )
                                    if "no_exp" in knobs:
                                        nc.vector.memset(exps[hh][:, 2 * kj : 2 * kj + 2, :], 0.5)
                                    else:
                                        nc.scalar.activation(
                                            out=exps[hh][:, 2 * kj : 2 * kj + 2, :],
                                            in_=sc_ps[:].rearrange(
                                                "p (a b) -> p a b", a=2
                                            ),
                                            func=mybir.ActivationFunctionType.Exp,
                                            scale=pat_bc[:, h : h + 1],
                                        )
                                if apply_mask:
                                    nc.vector.tensor_tensor(
                                        out=exps[hh][:, 2 * kj : 2 * kj + 2, :],
                                        in0=exps[hh][:, 2 * kj : 2 * kj + 2, :],
                                        in1=mk_sb[
                                            :, 2 * kj : 2 * kj + 2,
                                            qt * 512 : (qt + 1) * 512,
                                        ],
                                        op=mybir.AluOpType.mult,
                                    )
                                for half in range(2 if "no_av" not in knobs else 0):
                                    kt = 2 * kj + half
                                    nc.tensor.matmul(
                                        out=cts[hh][:],
                                        lhsT=vaug_sb[:, kt, 2 * hp + hh, :],
                                        rhs=exps[hh][:, kt, :],
                                        start=(kt == 0),
                                        stop=(kt == KT - 1),
                                    )
                        bcs = []
                        for hh in range(2):
                            h = 2 * hp + hh
                            ct_ps = cts[hh]
                            if "no_av" in knobs:
                                nc.vector.memset(ct_ps[:], 1.0)
                            sum_b = bcpool.tile([1, 512], BF16, tag="sumb")
                            nc.vector.tensor_copy(out=sum_b[:], in_=ct_ps[HD : HD + 1, :])
                            bc_ps = psB.tile([128, 512], F32, tag="b")
                            nc.tensor.matmul(
                                out=bc_ps[:], lhsT=ones128[:], rhs=sum_b[:],
                                start=True, stop=True,
                            )
                            bc_f = bcpool.tile([128, 512], F32, tag="bcf")
                            nc.vector.reciprocal(out=bc_f[:], in_=bc_ps[:])
                            bc_bf = bcpool.tile([128, 512], BF16, tag="bcbf")
                            nc.vector.tensor_copy(out=bc_bf[:], in_=bc_f[:])
                            # context normalize -> ctn[dlocal, tokens]
                            nc.vector.tensor_tensor(
                                out=ctn_sb[
                                    hh * HD : (hh + 1) * HD, hp,
                                    qt * 512 : (qt + 1) * 512,
                                ],
                                in0=ct_ps[0:HD, :],
                                in1=bc_bf[0:HD, :],
                                op=mybir.AluOpType.mult,
                            )
                            bcs.append(bc_bf)
                        for hh in range(2):
                            h = 2 * hp + hh
                            exp_sb = exps[hh]
                            bc_bf = bcs[hh]
                            # attn normalize + stage + DMA out
                            attn_st = stpool.tile([128, KT, 512], BF16, tag="st")
                            if "no_norm" not in knobs:
                                for kt in range(KT):
                                    nc.vector.tensor_tensor(
                                        out=attn_st[:, kt, :],
                                        in0=exp_sb[:, kt, :],
                                        in1=bc_bf[:],
                                        op=mybir.AluOpType.mult,
                                    )
                            else:
                                nc.vector.tensor_copy(out=attn_st[:, 0, :], in_=exp_sb[:, 0, :])
                            if "no_attn_dma" not in knobs and "no_norm" not in knobs:
                                nc.gpsimd.dma_start(
                                    out=attn_ext[h, qt], in_=attn_st[:]
                                )

                    # ---- output projection, delayed one qt for overlap ----
                    for tt4 in range(4 if qt > 0 else 0):
                        tt = (qt - 1) * 4 + tt4
                        ostage = opool.tile([128, HS], F32, tag="ost")
                        for oh in range(2):
                            o_ps = psB.tile([128, 512], F32, tag="b")
                            for pt in range(2):
                                nc.tensor.matmul(
                                    out=o_ps[:],
                                    lhsT=ctn_sb[:, pt, tt * 128 : (tt + 1) * 128],
                                    rhs=wo_sb[:, pt, oh * 512 : (oh + 1) * 512],
                                    start=(pt == 0),
                                    stop=(pt == 1),
                                )
                            nc.scalar.copy(
                                out=ostage[:, oh * 512 : (oh + 1) * 512], in_=o_ps[:]
                            )
                        nc.gpsimd.dma_start(
                            out=out_ext[tt * 128 : (tt + 1) * 128, :], in_=ostage[:]
                        )

                for tt4 in range(4):
                    tt = (QT - 1) * 4 + tt4
                    ostage = opool.tile([128, HS], F32, tag="ost")
                    for oh in range(2):
                        o_ps = psB.tile([128, 512], F32, tag="b")
                        for pt in range(2):
                            nc.tensor.matmul(
                                out=o_ps[:],
                                lhsT=ctn_sb[:, pt, tt * 128 : (tt + 1) * 128],
                                rhs=wo_sb[:, pt, oh * 512 : (oh + 1) * 512],
                                start=(pt == 0),
                                stop=(pt == 1),
                            )
                        if oh == 0:
                            nc.scalar.copy(
                                out=ostage[:, oh * 512 : (oh + 1) * 512], in_=o_ps[:]
                            )
                        else:
                            nc.vector.tensor_copy(
                                out=ostage[:, oh * 512 : (oh + 1) * 512], in_=o_ps[:]
                            )
                    nc.gpsimd.dma_start(
                        out=out_ext[tt * 128 : (tt + 1) * 128, :], in_=ostage[:]
                    )

    if split_waits:
        _split_excess_waits(nc)
    return nc


def _prep_inputs(query, key, value, attention_mask, Wq, bq, Wk, bk, Wv, bv,
                 Wo, bo, Wp1, bp1, Wp2, bp2, patterns, apply_mask):
    bf = ml_dtypes.bfloat16
    f32 = np.float32

    def arr_x(x_b):  # [S, HS] -> [128, CT, S] bf16 of x_b.T
        t = np.ascontiguousarray(x_b.T.reshape(CT, 128, S).transpose(1, 0, 2))
        return t.astype(bf)

    def arr_w(W, g):  # [HS, HS] -> [128, CT, DL] col slice
        Wg = W[:, g * DL : (g + 1) * DL]
        return np.ascontiguousarray(
            Wg.reshape(CT, 128, DL).transpose(1, 0, 2)
        ).astype(bf)

    xqs = [arr_x(np.asarray(query[b], f32)) for b in range(B)]
    xks = [arr_x(np.asarray(key[b], f32)) for b in range(B)]
    xvs = [arr_x(np.asarray(value[b], f32)) for b in range(B)]
    wp1_a = np.ascontiguousarray(
        (np.asarray(Wp1, f32) / S).reshape(CT, 128, 512).transpose(1, 0, 2)
    ).astype(bf)
    bp1_a = np.ascontiguousarray(np.asarray(bp1, f32).reshape(4, 128).T)
    wp2_a = np.ascontiguousarray(
        np.asarray(Wp2, f32).reshape(4, 128, NP).transpose(1, 0, 2)
    ).astype(bf)
    bp2_a = np.asarray(bp2, f32).reshape(NP, 1).copy()
    patterns_r = np.asarray(patterns, f32).reshape(NP, NH)

    if apply_mask:
        mks = []
        for b in range(B):
            mt = np.asarray(attention_mask[b], f32).T  # [k, q]
            mks.append(
                np.ascontiguousarray(
                    mt.reshape(KT, 128, S).transpose(1, 0, 2)
                ).astype(bf)
            )

    in_maps = []
    for c in range(N_CORES):
        b, g = c // 4, c % 4
        im = {
            "xq": xqs[b], "xk": xks[b], "xv": xvs[b],
            "wq": arr_w(np.asarray(Wq, f32), g),
            "wk": arr_w(np.asarray(Wk, f32), g),
            "wv": arr_w(np.asarray(Wv, f32), g),
            "wo": np.ascontiguousarray(
                np.asarray(Wo, f32)[g * DL : (g + 1) * DL, :]
                .reshape(2, 128, HS).transpose(1, 0, 2)
            ).astype(bf),
            "bq_c": np.ascontiguousarray(
                np.asarray(bq, f32)[g * DL : (g + 1) * DL].reshape(2, 128).T
            ),
            "bk_c": np.ascontiguousarray(
                np.asarray(bk, f32)[g * DL : (g + 1) * DL].reshape(2, 128).T
            ),
            "bv_b": np.ascontiguousarray(
                np.broadcast_to(
                    np.asarray(bv, f32)[g * DL : (g + 1) * DL].reshape(1, 4, 64),
                    (128, 4, 64),
                )
            ),
            "wp1": wp1_a, "bp1_c": bp1_a, "wp2": wp2_a, "bp2_c": bp2_a,
            "patm": np.ascontiguousarray(
                patterns_r[:, g * GH : (g + 1) * GH] * SCALE
            ).astype(bf),
        }
        if apply_mask:
            im["mk"] = mks[b]
        in_maps.append(im)
    return in_maps


def kernel(**inputs):
    attention_mask = np.asarray(inputs["attention_mask"])
    apply_mask = not bool(attention_mask.all())

    if apply_mask not in _cache:
        _cache[apply_mask] = _build(apply_mask)
    nc = _cache[apply_mask]

    in_maps = _prep_inputs(
        inputs["query"], inputs["key"], inputs["value"], attention_mask,
        inputs["Wq"], inputs["bq"], inputs["Wk"], inputs["bk"],
        inputs["Wv"], inputs["bv"], inputs["Wo"], inputs["bo"],
        inputs["Wp1"], inputs["bp1"], inputs["Wp2"], inputs["bp2"],
        inputs["patterns"], apply_mask,
    )

    res = run_bass_kernel_spmd(nc, in_maps, core_ids=list(range(N_CORES)))

    f32 = np.float32
    output = np.zeros((B, S, HS), f32)
    attn = np.empty((B, NH, S, S), f32)
    for c in range(N_CORES):
        b, g = c // 4, c % 4
        r = res.results[c]
        output[b] += np.asarray(r["out_part"], f32)
        at = np.asarray(r["attn_t"])  # [GH, QT, 128(p), KT, 512(ql)]
        attn[b, g * GH : (g + 1) * GH] = (
            at.transpose(0, 1, 4, 3, 2).reshape(GH, S, S).astype(f32)
        )
    output += np.asarray(inputs["bo"], f32)[None, None, :]
    return output, attn
